# revision 17
# baseline (speedup 1.0000x reference)
"""MLA forward on 8 TRN2 cores — v1 structure + safe v2 improvements.

2 (batch) x 4 (head-group) grid; latent path replicated per batch group
(no collectives). vs v1: Q/K head dims stored parity-swapped so k-rope/
k-nope transposes are 2-head-wide full-128 transposes; softmax 1/l is
broadcast with a K=1 PE outer product instead of GpSimd; psS 4 bufs;
e=1 x-chunk hoisted ahead of the weight stream.
"""

import sys

for _p in ("/opt/trn_rl_repo",):
    if _p not in sys.path:
        sys.path.insert(0, _p)

import math
from contextlib import ExitStack

import ml_dtypes
import numpy as np

import concourse.bass as bass
import concourse.mybir as mybir
import concourse.tile as tile
from concourse import bacc
from concourse.bass_utils import run_bass_kernel_spmd

F32 = mybir.dt.float32
BF16 = mybir.dt.bfloat16
BF = ml_dtypes.bfloat16

B, S, D = 2, 2048, 2048
H = 16
HD = 128
ROPE = 64
NOPE = 64
LAT = 512
EPS = 1e-6
ROPE_BASE = 10000.0

H_LOC = 4
N_CORES = 8
DLOC = H_LOC * HD

ST_N = S // 128
KT_N = D // 128
QB = 512
NB = 512

A_QW = H_LOC * HD             # 512
A_RW = H_LOC * ROPE           # 256
A_W = A_QW + A_RW + LAT       # 1280
KV_W = H_LOC * NOPE + H_LOC * HD   # 768

X8_CHUNK = 256
X8_N = S // X8_CHUNK
ST_PER_CHUNK = X8_CHUNK // 128

MULT = mybir.AluOpType.mult
ADD = mybir.AluOpType.add
SUB = mybir.AluOpType.subtract
EXPF = mybir.ActivationFunctionType.Exp
SQRTF = mybir.ActivationFunctionType.Sqrt
SQF = mybir.ActivationFunctionType.Square
AXX = mybir.AxisListType.X
AXXY = mybir.AxisListType.XY

_PROGRAM_CACHE = {}


def _build_program():
    nc = bacc.Bacc(None, target_bir_lowering=False, debug=True)

    xT8 = nc.dram_tensor("xT8", [X8_N, 128, KT_N, X8_CHUNK], BF16,
                         kind="ExternalInput")
    w_a = nc.dram_tensor("w_a", [D, A_W], BF16, kind="ExternalInput")
    w_up = nc.dram_tensor("w_up", [LAT, KV_W], BF16, kind="ExternalInput")
    w_p = nc.dram_tensor("w_p", [DLOC, D], BF16, kind="ExternalInput")
    cos4 = nc.dram_tensor("cos4", [128, ST_N, H_LOC, ROPE // 2], BF16,
                          kind="ExternalInput")
    sin4 = nc.dram_tensor("sin4", [128, ST_N, H_LOC, ROPE // 2], BF16,
                          kind="ExternalInput")
    masks = nc.dram_tensor("masks", [128, 4, QB], BF16, kind="ExternalInput")
    gain13 = nc.dram_tensor("gain13", [128, 13], F32, kind="ExternalInput")
    ident_in = nc.dram_tensor("ident_in", [128, 128], BF16, kind="ExternalInput")
    out = nc.dram_tensor("out", [S, D], BF16, kind="ExternalOutput")

    with tile.TileContext(nc) as tc, ExitStack() as top:
        const = top.enter_context(tc.tile_pool(name="const", bufs=1))
        big = top.enter_context(tc.tile_pool(name="big", bufs=1))

        wa_sb = const.tile([128, KT_N, A_W], BF16)
        wa_r = w_a[:].rearrange("(k p) n -> p k n", p=128)
        wup_sb = const.tile([128, LAT // 128, KV_W], BF16)
        cos_sb = const.tile([128, ST_N, H_LOC, ROPE // 2], BF16)
        sin_sb = const.tile([128, ST_N, H_LOC, ROPE // 2], BF16)
        gain_sb = const.tile([128, 13], F32)
        ident_sb = const.tile([128, 128], BF16)
        eps_sb = const.tile([128, 1], F32)
        nc.vector.memset(eps_sb[:], EPS)
        ones_col = const.tile([128, 1], BF16)
        nc.vector.memset(ones_col[:], 1.0)
        ones_row = const.tile([1, 128], BF16)
        nc.vector.memset(ones_row[:], 1.0)
        mask_sb = const.tile([128, 4, QB], BF16)
        wp_sb = const.tile([128, H_LOC, D], BF16)

        # parity-swapped per-head dim order: even heads [rope|nope], odd
        # heads [nope|rope] (matched between QT and KT)
        QT = big.tile([128, H_LOC, S], BF16)
        KT = big.tile([128, H_LOC, S], BF16)
        V = big.tile([128, ST_N, H_LOC * HD], BF16)

        p12 = ExitStack()
        ckvT_pool = p12.enter_context(tc.tile_pool(name="ckvT_pool", bufs=1))
        x8p = p12.enter_context(tc.tile_pool(name="x8p", bufs=2))
        xq0 = x8p.tile([128, KT_N, X8_CHUNK], BF16, tag="x8")
        nc.sync.dma_start(out=xq0[:, 0:1, :], in_=xT8[0, :, 0:1, :])
        nc.sync.dma_start(out=wa_sb[:, 0, 1024:1280], in_=wa_r[:, 0, 1024:1280])
        nc.sync.dma_start(out=xq0[:, 1:4, :], in_=xT8[0, :, 1:4, :])
        nc.sync.dma_start(out=wa_sb[:, 0, 0:1024], in_=wa_r[:, 0, 0:1024])
        nc.sync.dma_start(out=xq0[:, 4:8, :], in_=xT8[0, :, 4:8, :])
        xq1 = x8p.tile([128, KT_N, X8_CHUNK], BF16, tag="x8")
        for kt in range(1, KT_N):
            nc.sync.dma_start(out=wa_sb[:, kt, :], in_=wa_r[:, kt, :])
            if kt == 7:
                nc.sync.dma_start(out=xq0[:, 8:, :], in_=xT8[0, :, 8:, :])
        # e=1 chunk right after the weight stream (ahead of the tables) so
        # s-tiles 2-3 never starve
        nc.sync.dma_start(out=xq1[:], in_=xT8[1])
        nc.sync.dma_start(out=cos_sb[:], in_=cos4[:])
        nc.sync.dma_start(out=sin_sb[:], in_=sin4[:])
        nc.sync.dma_start(out=gain_sb[:], in_=gain13[:])
        nc.sync.dma_start(out=ident_sb[:], in_=ident_in[:])
        wp_r = w_p[:].rearrange("(k p) n -> p k n", p=128)
        scr = p12.enter_context(tc.tile_pool(name="scr", bufs=2))
        jnk = p12.enter_context(tc.tile_pool(name="jnk", bufs=2))
        ckvT = ckvT_pool.tile([128, LAT // 128, S], BF16)

        RH = ROPE // 2

        def rsqrt_act(dst, src, n):
            nc.scalar.activation(dst, src, SQRTF, scale=1.0 / n, bias=eps_sb[:])
            nc.vector.reciprocal_approx_fast(out=dst, in_=dst)

        # ========== phase 1: GEMM-A + norms + rope, per s-tile ==========
        with (
            tc.tile_pool(name="psA", bufs=2, space="PSUM") as psA,
            tc.tile_pool(name="psT", bufs=2, space="PSUM") as psT,
        ):
            for e in range(X8_N):
                if e == 0:
                    xq = xq0
                elif e == 1:
                    xq = xq1
                else:
                    xq = x8p.tile([128, KT_N, X8_CHUNK], BF16, tag="x8")
                    nc.sync.dma_start(out=xq[:], in_=xT8[e])
                if e == 1:
                    nc.sync.dma_start(out=mask_sb[:], in_=masks[:])
                elif 2 <= e <= 5:
                    c0 = (e - 2) * 512
                    nc.sync.dma_start(
                        out=wp_sb[:, :, c0:c0 + 512], in_=wp_r[:, :, c0:c0 + 512])
                elif e == 6:
                    nc.sync.dma_start(
                        out=wup_sb[:],
                        in_=w_up[:].rearrange("(k p) n -> p k n", p=128))
                for st2 in range(ST_PER_CHUNK):
                    ST = e * ST_PER_CHUNK + st2
                    s0 = ST * 128
                    aps = psA.tile([128, A_W], F32, tag="A")
                    for kt in range(KT_N):
                        lhs = xq[:, kt, st2 * 128:(st2 + 1) * 128]
                        for c0, c1 in ((1024, 1280), (0, 512), (512, 1024)):
                            nc.tensor.matmul(
                                aps[:, c0:c1], lhs, wa_sb[:, kt, c0:c1],
                                start=(kt == 0), stop=(kt == KT_N - 1))

                    asb = scr.tile([128, A_W], F32, tag="asb")
                    nc.scalar.copy(asb[:], aps[:])
                    junk = jnk.tile([128, A_W], BF16, tag="junk")
                    nc.scalar.activation(junk[:], aps[:], SQF)
                    rs13 = scr.tile([128, 13], F32, tag="rs13")
                    nc.vector.tensor_reduce(
                        rs13[:, 0:12],
                        junk[:, 0:768].rearrange("p (g c) -> p g c", c=64),
                        AXX, ADD)
                    nc.vector.tensor_reduce(
                        rs13[:, 12:13],
                        junk[:, 768:1280].rearrange("p (g c) -> p g c", c=64),
                        AXXY, ADD)
                    rsqrt_act(rs13[:, 0:12], rs13[:, 0:12], 64)
                    rsqrt_act(rs13[:, 12:13], rs13[:, 12:13], LAT)
                    nc.vector.tensor_tensor(rs13[:], rs13[:], gain_sb[:], MULT)

                    nrm = scr.tile([128, 768], BF16, tag="nrm")
                    nc.vector.tensor_tensor(
                        nrm[:].rearrange("p (g c) -> p g c", c=64),
                        asb[:, 0:768].rearrange("p (g c) -> p g c", c=64),
                        rs13[:, 0:12].to_broadcast([128, 12, 64]), MULT)
                    cv = scr.tile([128, LAT], BF16, tag="cv")
                    nc.vector.tensor_scalar(
                        cv[:], asb[:, 768:1280], rs13[:, 12:13], None, MULT)

                    # ---- rope + parity-packed q/k assembly ----
                    nrmq = nrm[:, 0:512].rearrange("p (h t c) -> p h t c",
                                                   t=2, c=64)
                    qno = nrmq[:, :, 0, :]
                    qro = nrmq[:, :, 1, :]
                    kro = nrm[:, 512:768].rearrange("p (h c) -> p h c", c=64)
                    qc = scr.tile([128, H_LOC, HD], BF16, tag="qc")
                    krot = scr.tile([128, 2, HD], BF16, tag="krot")
                    cosv = cos_sb[:, ST]
                    sinv = sin_sb[:, ST]

                    qc5 = qc[:].rearrange("p (r q) c -> p r q c", q=2)
                    qno5 = qno.rearrange("p (r q) c -> p r q c", q=2)
                    nc.vector.tensor_copy(qc5[:, :, 0, 64:128], qno5[:, :, 0, :])
                    nc.vector.tensor_copy(qc5[:, :, 1, 0:64], qno5[:, :, 1, :])

                    def rope4(xr, t_tag):
                        t1 = scr.tile([128, H_LOC, RH], F32, tag=t_tag + "1")
                        t2 = scr.tile([128, H_LOC, RH], F32, tag=t_tag + "2")
                        t3 = scr.tile([128, H_LOC, RH], F32, tag=t_tag + "3")
                        t4 = scr.tile([128, H_LOC, RH], F32, tag=t_tag + "4")
                        nc.vector.tensor_tensor(t1[:], xr[:, :, 0:RH], cosv, MULT)
                        nc.vector.tensor_tensor(t2[:], xr[:, :, RH:], sinv, MULT)
                        nc.vector.tensor_tensor(t3[:], xr[:, :, RH:], cosv, MULT)
                        nc.vector.tensor_tensor(t4[:], xr[:, :, 0:RH], sinv, MULT)
                        return t1, t2, t3, t4

                    t1, t2, t3, t4 = rope4(qro, "tq")
                    t1v = t1[:].rearrange("p (r q) c -> p r q c", q=2)
                    t2v = t2[:].rearrange("p (r q) c -> p r q c", q=2)
                    t3v = t3[:].rearrange("p (r q) c -> p r q c", q=2)
                    t4v = t4[:].rearrange("p (r q) c -> p r q c", q=2)
                    nc.vector.tensor_tensor(
                        qc5[:, :, 0, 0:RH], t1v[:, :, 0, :], t2v[:, :, 0, :], ADD)
                    nc.vector.tensor_tensor(
                        qc5[:, :, 1, 64:64 + RH], t1v[:, :, 1, :], t2v[:, :, 1, :], ADD)
                    nc.vector.tensor_tensor(
                        qc5[:, :, 0, RH:64], t3v[:, :, 0, :], t4v[:, :, 0, :], SUB)
                    nc.vector.tensor_tensor(
                        qc5[:, :, 1, 64 + RH:128], t3v[:, :, 1, :], t4v[:, :, 1, :], SUB)

                    u1, u2, u3, u4 = rope4(kro, "tk")
                    krot5 = krot[:].rearrange("p r (q o c) -> p r q o c",
                                              q=2, o=2, c=RH)
                    u1v = u1[:].rearrange("p (r q) c -> p r q c", q=2)
                    u2v = u2[:].rearrange("p (r q) c -> p r q c", q=2)
                    u3v = u3[:].rearrange("p (r q) c -> p r q c", q=2)
                    u4v = u4[:].rearrange("p (r q) c -> p r q c", q=2)
                    nc.vector.tensor_tensor(
                        krot5[:, :, :, 0, :], u1v[:], u2v[:], ADD)
                    nc.vector.tensor_tensor(
                        krot5[:, :, :, 1, :], u3v[:], u4v[:], SUB)

                    for hh in range(H_LOC):
                        tq = psT.tile([128, 128], BF16, tag="tq")
                        nc.tensor.transpose(tq[:], qc[:, hh, :], ident_sb[:])
                        nc.scalar.copy(QT[:, hh, s0:s0 + 128], tq[:])
                    for pr in range(2):
                        tk = psT.tile([128, 128], BF16, tag="tq")
                        nc.tensor.transpose(tk[:], krot[:, pr, :], ident_sb[:])
                        nc.scalar.copy(
                            KT[0:64, 2 * pr, s0:s0 + 128], tk[0:64, :])
                        nc.vector.tensor_copy(
                            KT[64:128, 2 * pr + 1, s0:s0 + 128], tk[64:128, :])
                    for lt in range(LAT // 128):
                        tcv = psT.tile([128, 128], BF16, tag="tq")
                        nc.tensor.transpose(
                            tcv[:], cv[:, lt * 128:(lt + 1) * 128], ident_sb[:])
                        nc.scalar.copy(ckvT[:, lt, s0:s0 + 128], tcv[:])

        # ================= phase 2: GEMM-3 (kv_up) =================
        with (
            tc.tile_pool(name="psKV", bufs=2, space="PSUM") as psKV,
            tc.tile_pool(name="psT2", bufs=2, space="PSUM") as psT2,
        ):
            def kn_transposes(pknrm, ps0):
                for pr in range(2):
                    tkn = psT2.tile([128, 128], BF16, tag="tkn")
                    nc.tensor.transpose(tkn[:], pknrm[:, pr, :], ident_sb[:])
                    nc.scalar.copy(
                        KT[0:64, 2 * pr + 1, ps0:ps0 + 128], tkn[0:64, :])
                    nc.vector.tensor_copy(
                        KT[64:128, 2 * pr, ps0:ps0 + 128], tkn[64:128, :])

            prev_kn = None
            for ST in range(ST_N):
                s0 = ST * 128
                kvps = psKV.tile([128, KV_W], F32, tag="KV")
                for lt in range(LAT // 128):
                    lhs = ckvT[:, lt, s0:s0 + 128]
                    for c0, c1 in ((0, 512), (512, 768)):
                        nc.tensor.matmul(
                            kvps[:, c0:c1], lhs, wup_sb[:, lt, c0:c1],
                            start=(lt == 0), stop=(lt == LAT // 128 - 1))
                # previous tile's transposes here: their DVE chain has had a
                # whole kv_up GEMM of PE time to finish, so PE never waits
                if prev_kn is not None:
                    kn_transposes(*prev_kn)
                kvev = scr.tile([128, 256], F32, tag="kvev")
                nc.scalar.copy(kvev[:], kvps[:, 0:256])
                junkk = jnk.tile([128, 256], BF16, tag="junkk")
                nc.scalar.activation(junkk[:], kvps[:, 0:256], SQF)
                rsk = scr.tile([128, 4], F32, tag="rsk")
                nc.vector.tensor_reduce(
                    rsk[:], junkk[:].rearrange("p (g c) -> p g c", c=64),
                    AXX, ADD)
                rsqrt_act(rsk[:], rsk[:], 64)
                # pair layout [h_odd nope | h_even nope]: one 128-wide
                # transpose serves two heads
                knrm = scr.tile([128, 2, 128], BF16, tag="knrm")
                for hh in range(H_LOC):
                    pr, odd = hh // 2, hh % 2
                    dst = knrm[:, pr, 0:64] if odd else knrm[:, pr, 64:128]
                    nc.vector.tensor_scalar(
                        dst, kvev[:, hh * 64:(hh + 1) * 64],
                        rsk[:, hh:hh + 1], None, MULT)
                prev_kn = (knrm, s0)
                nc.scalar.copy(V[:, ST, :], kvps[:, H_LOC * NOPE:KV_W])
            if prev_kn is not None:
                kn_transposes(*prev_kn)

        p12.close()

        # ==== phase 3: attention + out projection (interleaved per q-block) ==
        yT_pool = top.enter_context(tc.tile_pool(name="yT_pool", bufs=1))
        yT = yT_pool.tile([128, H_LOC, S], BF16)
        inv_sqrt_hd = 1.0 / math.sqrt(HD)
        LAG = 7
        with (
            tc.tile_pool(name="pP", bufs=18) as pP,
            tc.tile_pool(name="pT", bufs=14) as pTs,
            tc.tile_pool(name="pU", bufs=3) as pU,
            tc.tile_pool(name="pR", bufs=2) as pR,
            tc.tile_pool(name="pO", bufs=4) as pO,
            tc.tile_pool(name="psS", bufs=4, space="PSUM") as psS,
            tc.tile_pool(name="psY", bufs=2, space="PSUM") as psY,
            tc.tile_pool(name="psO", bufs=2, space="PSUM") as psO,
        ):
            def tree_add(a, b):
                t = pTs.tile([128, QB], BF16, tag="tsum")
                nc.vector.tensor_tensor(t[:], a[:], b[:], ADD)
                return t

            def proj_steps(s0, order=(0, 1, 2, 3)):
                for nb in range(D // NB):
                    ot = psO.tile([128, NB], F32, tag="O")
                    for i, hh in enumerate(order):
                        nc.tensor.matmul(
                            ot[:], yT[:, hh, s0:s0 + 128],
                            wp_sb[:, hh, nb * NB:(nb + 1) * NB],
                            start=(i == 0), stop=(i == H_LOC - 1))
                        if i < H_LOC - 1:
                            yield
                    osb = pO.tile([128, NB], BF16, tag="osb")
                    if nb % 2 == 0:
                        nc.scalar.copy(osb[:], ot[:])
                    else:
                        nc.vector.tensor_copy(osb[:], ot[:])
                    nc.sync.dma_start(
                        out=out[s0:s0 + 128, nb * NB:(nb + 1) * NB],
                        in_=osb[:])
                    yield

            def drain(gen, frac=1.0):
                if gen is None:
                    return None
                n = int(16 * frac + 0.5)
                try:
                    while gen[1] < n:
                        next(gen[0])
                        gen[1] += 1
                except StopIteration:
                    return None
                return gen

            # Softmax 1/l epilogue is DEFERRED into the next head's score
            # stream: l = ones-matmul colsum of the tree root, DVE
            # reciprocal, then a K=1 outer-product matmul broadcasts 1/l
            # across partitions. Deferring hides the DVE latency behind PE
            # work so neither epilogue matmul ever stalls the PE.
            pending = [None]

            def emit_pending(upto):
                pd = pending[0]
                if pd is None:
                    return
                if pd["stage"] == 0 and upto >= 0:
                    lps = psS.tile([128, QB], F32, tag="Ssc")
                    nc.tensor.matmul(
                        lps[0:1, :], ones_col[:], pd["root"][:],
                        start=True, stop=True)
                    r1f = pR.tile([1, QB], F32, tag="r1f")
                    nc.vector.reciprocal_approx_fast(
                        out=r1f[:], in_=lps[0:1, :])
                    r1 = pR.tile([1, QB], BF16, tag="r1")
                    nc.vector.tensor_copy(r1[:], r1f[:])
                    pd["r1"] = r1
                    pd["stage"] = 1
                if pd["stage"] == 1 and upto >= 1:
                    bps = psS.tile([128, QB], F32, tag="Ssc")
                    nc.tensor.matmul(
                        bps[:], ones_row[:], pd["r1"][:],
                        start=True, stop=True)
                    nc.vector.tensor_tensor(
                        yT[:, pd["h"], pd["q0"]:pd["q0"] + QB],
                        pd["yU"][:], bps[:], MULT)
                    pending[0] = None

            for j in range(S // QB):
                q0 = j * QB
                nkt = (q0 + QB) // 128
                horder = range(H_LOC) if j < S // QB - 1 else (3, 2, 1, 0)
                for hi, h in enumerate(horder):
                    pj = [proj_steps((j - 1) * QB + h * 128), 0] if j >= 1 \
                        else None
                    # the first head of a block still has the PREVIOUS
                    # block's last-head scale pending (emitted at kt 3/7):
                    # its proj drain must not start before that flush
                    delay_pj = (hi == 0)
                    yps = psY.tile([128, QB], F32, tag="Y")
                    Ptiles = []
                    counter = []
                    for kt in range(nkt):
                        sps = psS.tile([128, QB], F32, tag="Ssc")
                        nc.tensor.matmul(
                            sps[:], KT[:, h, kt * 128:(kt + 1) * 128],
                            QT[:, h, q0:q0 + QB], start=True, stop=True)
                        P = pP.tile([128, QB], BF16, tag="P")
                        nc.scalar.activation(
                            P[:], sps[:], EXPF, scale=inv_sqrt_hd)
                        d_idx = kt - (q0 // 128)
                        if d_idx >= 0:
                            nc.vector.tensor_tensor(
                                P[:], P[:], mask_sb[:, d_idx, :], MULT)
                        Ptiles.append(P)
                        t, lv = P, 0
                        while counter and counter[-1][0] == lv:
                            t = tree_add(counter.pop()[1], t)
                            lv += 1
                        counter.append((lv, t))
                        if kt >= LAG:
                            k2 = kt - LAG
                            nc.tensor.matmul(
                                yps[:], V[:, k2, h * HD:(h + 1) * HD],
                                Ptiles[k2][:],
                                start=(k2 == 0), stop=(k2 == nkt - 1))
                        if kt == 3:
                            emit_pending(0)
                        elif kt == 7:
                            emit_pending(1)
                        if delay_pj:
                            if kt >= 7:
                                pj = drain(
                                    pj, (kt - 6) / max(1, nkt - 7))
                        else:
                            pj = drain(pj, (kt + 1) / nkt)
                    for k2 in range(max(0, nkt - LAG), nkt):
                        nc.tensor.matmul(
                            yps[:], V[:, k2, h * HD:(h + 1) * HD],
                            Ptiles[k2][:],
                            start=(k2 == 0), stop=(k2 == nkt - 1))
                    yU = pU.tile([128, QB], BF16, tag="yU")
                    nc.vector.tensor_copy(yU[:], yps[:])
                    root = counter.pop()[1]
                    while counter:
                        root = tree_add(counter.pop()[1], root)
                    emit_pending(1)
                    pj = drain(pj)
                    pending[0] = {"stage": 0, "root": root, "yU": yU,
                                  "h": h, "q0": q0}
            for st2 in range(QB // 128):
                pj = [proj_steps(3 * QB + st2 * 128, order=(3, 2, 1, 0)), 0]
                if st2 == 0 and pending[0] is not None:
                    # steps 0-2 read yT[3..1] (already scaled); the step
                    # reading yT[0] comes only after the flush completes
                    pj = drain(pj, 2 / 16)
                    emit_pending(0)
                    pj = drain(pj, 3 / 16)
                    emit_pending(1)
                drain(pj)
    nc.compile()
    return nc


def _prep_inputs(x, w_q_krope, w_kv_down, w_kv_up, w_proj, q_gain):
    inv_freq = ROPE_BASE ** (-np.arange(0, ROPE, 2, dtype=np.float32) / ROPE)
    t = np.arange(S, dtype=np.float32)
    freqs = np.outer(t, inv_freq)
    def _pmajor(tab):
        full = np.broadcast_to(tab[:, None, :], (S, H_LOC, ROPE // 2))
        return np.ascontiguousarray(
            full.reshape(ST_N, 128, H_LOC, ROPE // 2).transpose(1, 0, 2, 3)
        ).astype(BF)

    cos4 = _pmajor(np.cos(freqs))
    sin4 = _pmajor(np.sin(freqs))

    kk = np.arange(128)[:, None, None]
    dd = np.arange(4)[None, :, None]
    qq = np.arange(QB)[None, None, :]
    masks = (kk + 128 * dd <= qq).astype(BF)

    ident_in = np.eye(128, dtype=np.float32).astype(BF)

    xT_chunks = []
    for b in range(B):
        xT = np.ascontiguousarray(x[b].T).astype(BF)
        xT_chunks.append(np.ascontiguousarray(
            xT.reshape(KT_N, 128, X8_N, X8_CHUNK).transpose(2, 1, 0, 3)))

    in_maps = []
    for c in range(N_CORES):
        b = c // H_LOC
        hg = c % H_LOC
        heads = [hg * H_LOC + i for i in range(H_LOC)]
        w_a = np.concatenate(
            [w_q_krope[:, h * HD:(h + 1) * HD] for h in heads]
            + [w_q_krope[:, D + h * ROPE:D + (h + 1) * ROPE] for h in heads]
            + [w_kv_down], axis=1).astype(BF)
        w_up = np.concatenate(
            [w_kv_up[:, h * NOPE:(h + 1) * NOPE] for h in heads]
            + [w_kv_up[:, NOPE * H + h * HD:NOPE * H + (h + 1) * HD]
               for h in heads], axis=1).astype(BF)
        w_p = w_proj[hg * DLOC:(hg + 1) * DLOC, :].astype(BF)
        g = q_gain[heads].astype(np.float32)
        g13 = np.concatenate([np.repeat(g, 2), np.ones(5, np.float32)])
        gain13 = np.ascontiguousarray(
            np.broadcast_to(g13[None, :], (128, 13))).astype(np.float32)
        in_maps.append({
            "xT8": xT_chunks[b],
            "w_a": np.ascontiguousarray(w_a),
            "w_up": np.ascontiguousarray(w_up),
            "w_p": np.ascontiguousarray(w_p),
            "cos4": cos4, "sin4": sin4, "masks": masks,
            "gain13": gain13, "ident_in": ident_in,
        })
    return in_maps


def kernel(x, w_q_krope, w_kv_down, w_kv_up, w_proj, q_gain, **_unused):
    x = np.asarray(x, dtype=np.float32)
    w_q_krope = np.asarray(w_q_krope, dtype=np.float32)
    w_kv_down = np.asarray(w_kv_down, dtype=np.float32)
    w_kv_up = np.asarray(w_kv_up, dtype=np.float32)
    w_proj = np.asarray(w_proj, dtype=np.float32)
    q_gain = np.asarray(q_gain, dtype=np.float32)

    if "nc" not in _PROGRAM_CACHE:
        _PROGRAM_CACHE["nc"] = _build_program()
    nc = _PROGRAM_CACHE["nc"]

    in_maps = _prep_inputs(x, w_q_krope, w_kv_down, w_kv_up, w_proj, q_gain)
    res = run_bass_kernel_spmd(nc, in_maps, list(range(N_CORES)))

    out = np.zeros((B, S, D), dtype=np.float32)
    for c in range(N_CORES):
        out[c // H_LOC] += res.results[c]["out"].astype(np.float32)
    return out
